# revision 4
# baseline (speedup 1.0000x reference)
"""Trainium2 Bass kernel for nn_NeuralSurface (8-layer MLP SDF with harmonic
embedding + skip concat), data-parallel over 8 NeuronCores.

Layout: activations transposed in SBUF ([feat, points]), weights stationary
fp16. Two point-pairs (A/B, 1024 pts each) interleaved per layer so the PE
never waits on ReLU drains. PSUM organized as [128, 1024] two-bank tiles per
M-half (same bias column) so each drain is a single fused ACT/DVE op. Layer-0
and layer-4 embedding chunks (K=39->64) run as row-tiled matmul pairs at array
rows 0/64 (concurrent). Harmonic sin/cos via ScalarE Sin LUT after GpSimd
range reduction (magic-number round-to-nearest). Finals col-packed 4-per-bank.
PE warmup matmuls cover the input-DMA head so HAM is at 2.4GHz from the first
real matmul.
"""

import numpy as np

import concourse.bacc as bacc
import concourse.mybir as mybir
import concourse.tile as tile
from concourse.bass_utils import run_bass_kernel_spmd

AF = mybir.ActivationFunctionType
ALU = mybir.AluOpType
F32 = mybir.dt.float32
F16 = mybir.dt.float16

N_CORES = 8
N = 262144
NPC = N // N_CORES  # 32768 points per core
NT = 512  # points per n-tile (PSUM bank / moving-operand limit)
W = 2 * NT  # pair width (1024 points)
NPAIR = NPC // W  # 32
NGROUP = NPAIR // 2  # 16 (two pairs A/B interleaved per group)
H = 256
E = 39
NHARM = 6
TWO_PI = float(2.0 * np.pi)
MAGIC = float(1.5 * 2.0**23)  # round-to-nearest via (x + M) - M
N_WARM = 26  # PE warmup matmuls (N=256) covering the input-DMA head

_CACHED = {}


def bass_ts(i, size):
    return slice(i * size, (i + 1) * size)


def _build():
    nc = bacc.Bacc("TRN2")

    rep6 = nc.dram_tensor("rep6", [128, NPC], F32, kind="ExternalInput").ap()
    ptsh = nc.dram_tensor("ptsh", [3, NPC], F16, kind="ExternalInput").ap()
    # w0/w4e packed for 64-row array tiling: rows 0:39 = M-half 0, rows
    # 64:103 = M-half 1 (matching emb duplicated at partitions 64:103).
    w0h = nc.dram_tensor("w0h", [128, 128], F16, kind="ExternalInput").ap()
    w4eh = nc.dram_tensor("w4eh", [128, 128], F16, kind="ExternalInput").ap()
    wkh = {
        i: nc.dram_tensor(f"w{i}h", [H, H], F16, kind="ExternalInput").ap()
        for i in (1, 2, 3, 5, 6, 7)
    }
    w4ah = nc.dram_tensor("w4ah", [128, H], F16, kind="ExternalInput").ap()
    w4bh = nc.dram_tensor("w4bh", [128, H], F16, kind="ExternalInput").ap()
    wsdfh = nc.dram_tensor("wsdfh", [H, 1], F16, kind="ExternalInput").ap()
    bmat = nc.dram_tensor("bmat", [128, 16], F32, kind="ExternalInput").ap()
    bsdf1 = nc.dram_tensor("bsdf1", [128, 1], F32, kind="ExternalInput").ap()
    # 2-D output (1-D ExternalOutput tensors fail NEFF load under bass2jax)
    out_o = nc.dram_tensor("out_o", [NPC // NT, NT], F32, kind="ExternalOutput").ap()

    with tile.TileContext(nc) as tc:
        with (
            tc.tile_pool(name="wp", bufs=1) as wp,
            tc.tile_pool(name="raw", bufs=3) as raw,
            tc.tile_pool(name="embp", bufs=4) as embp,
            tc.tile_pool(name="hp", bufs=6) as hp,
            tc.tile_pool(name="op", bufs=2) as op_,
            tc.tile_pool(name="ppd", bufs=3, space="PSUM") as ppd,
            tc.tile_pool(name="pf", bufs=1, space="PSUM") as pf,
        ):
            # ---- early: weights needed first + warmup ----
            w0s = wp.tile_from(w0h, name="w0s")  # [128, 128] packed
            bsdfs = wp.tile_from(bsdf1, name="bsdfs")  # [128, 1]
            zcol = wp.tile([128, 1], F32, name="zcol")
            nc.gpsimd.memset(zcol, 0.0)
            zwarm = wp.tile([128, 256], F16, name="zwarm")
            nc.gpsimd.memset(zwarm, 0.0)

            ps_warm = pf.tile([128, 256], F32, tag="warm")
            for _ in range(N_WARM):
                nc.tensor.matmul(
                    ps_warm, zwarm[:, 0:128], zwarm,
                    start=True, stop=True, skip_group_check=True,
                )

            # ---- embedding prep (t0 DMA ordered ahead of bulk weights) ----
            def emb_prep(p):
                s = p * W
                t0 = raw.tile([128, W], F32, tag="t0")
                nc.sync.dma_start(out=t0, in_=rep6[:, s:s + W])
                rr = raw.tile([128, W], F32, tag="rr")
                nc.gpsimd.tensor_scalar(
                    rr, t0, MAGIC, MAGIC, op0=ALU.add, op1=ALU.subtract
                )
                ys = raw.tile([128, W], F32, tag="ys")
                nc.gpsimd.tensor_tensor(out=ys, in0=t0, in1=rr, op=ALU.subtract)
                emb = embp.tile([128, W], F16, tag="emb")
                nc.scalar.activation(emb, ys, AF.Sin, bias=zcol, scale=TWO_PI)
                nc.sync.dma_start(out=emb[36:39, :], in_=ptsh[:, s:s + W])
                nc.sync.dma_start(out=emb[100:103, :], in_=ptsh[:, s:s + W])
                return emb

            emb_next = [emb_prep(0), emb_prep(1)]

            # ---- bulk one-time weight loads ----
            bms = wp.tile_from(bmat, name="bms")  # [128, 16]
            wks = {
                i: (
                    wp.tile_from(wkh[i][0:128, :], name=f"wks{i}a"),
                    wp.tile_from(wkh[i][128:256, :], name=f"wks{i}b"),
                )
                for i in (1, 2, 3, 5, 6, 7)
            }
            w4es = wp.tile_from(w4eh, name="w4es")  # [128, 128] packed
            w4as = wp.tile_from(w4ah, name="w4as")  # [128, 256]
            w4bs = wp.tile_from(w4bh, name="w4bs")
            wsdf_a = wp.tile_from(wsdfh[0:128, :], name="wsdf_a")  # [128, 1]
            wsdf_b = wp.tile_from(wsdfh[128:256, :], name="wsdf_b")

            # h slot layout: [m0hx0, m0hx1, m1hx0, m1hx1] -> slot = 2*m + hx;
            # so a fused [128, 1024] drain per M-half gets one bias column.
            def drain(li, m, ps, h):
                dst = h[:, bass_ts(m, W)]
                bias_ap = bms[:, li * 2 + m:li * 2 + m + 1]
                if m == 0:
                    nc.scalar.activation(dst, ps, AF.Relu, bias=bias_ap)
                else:
                    nc.vector.tensor_scalar(
                        dst, ps, bias_ap, 0.0, op0=ALU.add, op1=ALU.max
                    )

            def layer(li, emb, h_prev, h3):
                h = hp.tile([128, 2 * W], F16, tag="h")
                if li == 0:
                    ps = {m: ppd.tile([128, W], F32, tag="mm2", name=f"ps0_{m}")
                          for m in (0, 1)}
                    # row-tiled K=64 pairs at array rows 0/64 run concurrent
                    for hx in (0, 1):
                        for m in (0, 1):
                            nc.tensor.matmul(
                                ps[m][:, bass_ts(hx, NT)],
                                w0s[bass_ts(m, 64), :],
                                emb[bass_ts(m, 64), bass_ts(hx, NT)],
                                start=True, stop=True,
                                tile_position=(64 * m, 0),
                            )
                    drain(li, 0, ps[0], h)
                    drain(li, 1, ps[1], h)
                    return h
                if li == 4:
                    ps = {m: ppd.tile([128, W], F32, tag="mm2", name=f"ps4_{m}")
                          for m in (0, 1)}
                    for hx in (0, 1):
                        for m in (0, 1):
                            nc.tensor.matmul(
                                ps[m][:, bass_ts(hx, NT)],
                                w4es[bass_ts(m, 64), :],
                                emb[bass_ts(m, 64), bass_ts(hx, NT)],
                                start=True, stop=False,
                                tile_position=(64 * m, 0),
                                skip_group_check=True,
                            )
                    for m in (0, 1):
                        for hx in (0, 1):
                            nc.tensor.matmul(
                                ps[m][:, bass_ts(hx, NT)],
                                w4as[:, bass_ts(m, 128)],
                                h3[:, bass_ts(hx, NT)],
                                start=False, stop=False,
                                skip_group_check=True,
                            )
                            nc.tensor.matmul(
                                ps[m][:, bass_ts(hx, NT)],
                                w4bs[:, bass_ts(m, 128)],
                                h3[:, bass_ts(2 + hx, NT)],
                                start=False, stop=True,
                                skip_group_check=True,
                            )
                        drain(li, m, ps[m], h)
                    return h
                # middle layers: K=256 as two full 128-chunks
                wa, wb = wks[li]
                for m in (0, 1):
                    ps = ppd.tile([128, W], F32, tag="mm2")
                    for hx in (0, 1):
                        nc.tensor.matmul(
                            ps[:, bass_ts(hx, NT)],
                            wa[:, bass_ts(m, 128)],
                            h_prev[:, bass_ts(hx, NT)],
                            start=True, stop=False,
                        )
                        nc.tensor.matmul(
                            ps[:, bass_ts(hx, NT)],
                            wb[:, bass_ts(m, 128)],
                            h_prev[:, bass_ts(2 + hx, NT)],
                            start=False, stop=True,
                        )
                    drain(li, m, ps, h)
                return h

            for g in range(NGROUP):
                embA, embB = emb_next
                hA = hB = None
                h3A = h3B = None
                for li in range(8):
                    hA = layer(li, embA, hA, h3A)
                    hB = layer(li, embB, hB, h3B)
                    if li == 3:
                        h3A, h3B = hA, hB
                    if li == 4 and g + 1 < NGROUP:
                        # prefetch next group's embeddings mid-group so the
                        # Sin/DMA land before this group's finals
                        emb_next = [emb_prep(2 * g + 2), emb_prep(2 * g + 3)]

                # ---- finals: 4 outputs col-packed in one PSUM bank ----
                psf = pf.tile([128, NT], F32, tag="fin")
                ofin = op_.tile([97, NT], F32, tag="ofin")
                for j, (h7, hx) in enumerate(
                    [(hA, 0), (hA, 1), (hB, 0), (hB, 1)]
                ):
                    nc.tensor.matmul(
                        psf[32 * j:32 * j + 1, :], wsdf_a,
                        h7[:, bass_ts(hx, NT)],
                        start=True, stop=False, tile_position=(0, 32 * j),
                        skip_group_check=True,
                    )
                    nc.tensor.matmul(
                        psf[32 * j:32 * j + 1, :], wsdf_b,
                        h7[:, bass_ts(2 + hx, NT)],
                        start=False, stop=True, tile_position=(0, 32 * j),
                        skip_group_check=True,
                    )
                nc.scalar.activation(
                    ofin, psf[0:97, :], AF.Identity, bias=bsdfs[0:97, 0:1]
                )
                for j in range(4):
                    nc.sync.dma_start(
                        out=out_o[4 * g + j:4 * g + j + 1, :],
                        in_=ofin[32 * j:32 * j + 1, :],
                    )
    nc.compile()
    return nc


def _prep_maps(points, ws, bs, wsdf, bsdf):
    pts = np.ascontiguousarray(points, dtype=np.float32).reshape(N, 3)
    freqs = (2.0 ** np.arange(NHARM)).astype(np.float32)
    fcol18 = (np.repeat(freqs[None, :], 3, axis=0).reshape(18, 1) / TWO_PI).astype(
        np.float32
    )

    bmat = np.zeros((128, 16), dtype=np.float32)
    for i in range(8):
        for m in range(2):
            bmat[:, i * 2 + m] = bs[i][m * 128:(m + 1) * 128]

    # packed layer-0 / layer-4-emb weights: M-half 0 at rows 0:39, M-half 1
    # at rows 64:103 (for row-tiled matmuls at array rows 0/64)
    w0p = np.zeros((128, 128), dtype=np.float16)
    w0p[0:E, :] = ws[0][:, 0:128].astype(np.float16)
    w0p[64:64 + E, :] = ws[0][:, 128:256].astype(np.float16)
    w4ep = np.zeros((128, 128), dtype=np.float16)
    w4ep[0:E, :] = ws[4][0:E, 0:128].astype(np.float16)
    w4ep[64:64 + E, :] = ws[4][0:E, 128:256].astype(np.float16)
    common = {
        "w0h": w0p,
        "w4eh": w4ep,
        "w4ah": ws[4][E:E + 128, :].astype(np.float16),
        "w4bh": ws[4][E + 128:E + 256, :].astype(np.float16),
        "wsdfh": wsdf.astype(np.float16),
        "bmat": bmat,
        "bsdf1": np.full((128, 1), float(np.ravel(bsdf)[0]), dtype=np.float32),
    }
    for i in (1, 2, 3, 5, 6, 7):
        common[f"w{i}h"] = ws[i].astype(np.float16)

    in_maps = []
    for c in range(N_CORES):
        sl = pts[c * NPC:(c + 1) * NPC]  # [NPC, 3]
        ptsT = np.ascontiguousarray(sl.T)  # [3, NPC]
        rep3 = np.repeat(ptsT, NHARM, axis=0)  # [18, NPC]
        t18 = rep3 * fcol18  # x * 2^j / (2pi), exact fp32 scaling
        rep6 = np.zeros((128, NPC), dtype=np.float32)
        rep6[0:18], rep6[18:36] = t18, t18 + np.float32(0.25)
        rep6[64:82], rep6[82:100] = t18, t18 + np.float32(0.25)
        m = dict(common)
        m["rep6"] = rep6
        m["ptsh"] = ptsT.astype(np.float16)
        in_maps.append(m)
    return in_maps


def kernel(
    points, w0, b0, w1, b1, w2, b2, w3, b3, w4, b4, w5, b5, w6, b6, w7, b7,
    wsdf, bsdf,
):
    ws = [np.asarray(w, dtype=np.float32) for w in (w0, w1, w2, w3, w4, w5, w6, w7)]
    bs = [np.asarray(b, dtype=np.float32) for b in (b0, b1, b2, b3, b4, b5, b6, b7)]
    in_maps = _prep_maps(
        np.asarray(points), ws, bs,
        np.asarray(wsdf, dtype=np.float32), np.asarray(bsdf, dtype=np.float32),
    )

    if "nc" not in _CACHED:
        _CACHED["nc"] = _build()
    nc = _CACHED["nc"]

    res = run_bass_kernel_spmd(nc, in_maps, core_ids=list(range(N_CORES)))
    out = np.concatenate(
        [res.results[c]["out_o"] for c in range(N_CORES)], axis=0
    ).reshape(N, 1).astype(np.float32)
    return out


# revision 7
# speedup vs baseline: 1.3149x; 1.3149x over previous
"""Trainium2 Bass kernel for nn_NeuralSurface (8-layer MLP SDF with harmonic
embedding + skip concat), data-parallel over 8 NeuronCores.

Layout: activations transposed in SBUF ([feat, points]), weights stationary
fp16. Two point-pairs (A/B, 1024 pts each) interleaved per layer so the PE
never waits on ReLU drains. PSUM organized as [128, 1024] two-bank tiles per
M-half (same bias column) so each drain is a single fused ACT/DVE op. Layer-0
and layer-4 embedding chunks (K=39->64) run as row-tiled matmul pairs at array
rows 0/64 (concurrent). Harmonic sin/cos via ScalarE Sin LUT after GpSimd
range reduction (magic-number round-to-nearest). Finals col-packed 4-per-bank.
PE warmup matmuls cover the input-DMA head so HAM is at 2.4GHz from the first
real matmul.
"""

import numpy as np

import concourse.bacc as bacc
import concourse.mybir as mybir
import concourse.tile as tile
from concourse.bass_utils import run_bass_kernel_spmd

AF = mybir.ActivationFunctionType
ALU = mybir.AluOpType
F32 = mybir.dt.float32
F16 = mybir.dt.float16

N_CORES = 8
N = 262144
NPC = N // N_CORES  # 32768 points per core
NT = 512  # points per n-tile (PSUM bank / moving-operand limit)
W = 2 * NT  # pair width (1024 points)
NPAIR = NPC // W  # 32
NGROUP = NPAIR // 2  # 16 (two pairs A/B interleaved per group)
H = 256
E = 39
NHARM = 6
TWO_PI = float(2.0 * np.pi)
MAGIC = float(1.5 * 2.0**23)  # round-to-nearest via (x + M) - M
N_WARM = 26  # PE warmup matmuls (N=256) covering the input-DMA head

_CACHED = {}


def bass_ts(i, size):
    return slice(i * size, (i + 1) * size)


def _build():
    nc = bacc.Bacc("TRN2")

    rep6 = nc.dram_tensor("rep6", [128, NPC], F32, kind="ExternalInput").ap()
    ptsh = nc.dram_tensor("ptsh", [3, NPC], F16, kind="ExternalInput").ap()
    # w0/w4e packed for 64-row array tiling: rows 0:39 = M-half 0, rows
    # 64:103 = M-half 1 (matching emb duplicated at partitions 64:103).
    w0h = nc.dram_tensor("w0h", [128, 128], F16, kind="ExternalInput").ap()
    w4eh = nc.dram_tensor("w4eh", [128, 128], F16, kind="ExternalInput").ap()
    wkh = {
        i: nc.dram_tensor(f"w{i}h", [H, H], F16, kind="ExternalInput").ap()
        for i in (1, 2, 3, 5, 6, 7)
    }
    w4ah = nc.dram_tensor("w4ah", [128, H], F16, kind="ExternalInput").ap()
    w4bh = nc.dram_tensor("w4bh", [128, H], F16, kind="ExternalInput").ap()
    wsdfh = nc.dram_tensor("wsdfh", [H, 1], F16, kind="ExternalInput").ap()
    bmat = nc.dram_tensor("bmat", [128, 16], F32, kind="ExternalInput").ap()
    bsdf1 = nc.dram_tensor("bsdf1", [128, 1], F32, kind="ExternalInput").ap()
    # 2-D output (1-D ExternalOutput tensors fail NEFF load under bass2jax)
    out_o = nc.dram_tensor("out_o", [NPC // NT, NT], F32, kind="ExternalOutput").ap()

    with tile.TileContext(nc) as tc:
        with (
            tc.tile_pool(name="wp", bufs=1) as wp,
            tc.tile_pool(name="raw", bufs=3) as raw,
            tc.tile_pool(name="embp", bufs=4) as embp,
            tc.tile_pool(name="hp", bufs=6) as hp,
            tc.tile_pool(name="op", bufs=2) as op_,
            tc.tile_pool(name="ppd", bufs=3, space="PSUM") as ppd,
            tc.tile_pool(name="pf", bufs=1, space="PSUM") as pf,
        ):
            # ---- early: weights needed first + warmup ----
            w0s = wp.tile_from(w0h, name="w0s")  # [128, 128] packed
            bsdfs = wp.tile_from(bsdf1, name="bsdfs")  # [128, 1]
            zcol = wp.tile([128, 1], F32, name="zcol")
            nc.gpsimd.memset(zcol, 0.0)
            zwarm = wp.tile([128, 256], F16, name="zwarm")
            nc.gpsimd.memset(zwarm, 0.0)

            ps_warm = pf.tile([128, 256], F32, tag="warm")
            for _ in range(N_WARM):
                nc.tensor.matmul(
                    ps_warm, zwarm[:, 0:128], zwarm,
                    start=True, stop=True, skip_group_check=True,
                )

            # ---- embedding prep (t0 DMA ordered ahead of bulk weights) ----
            def emb_prep(p):
                # rep6 rows carry host-side range-reduced phases ys in
                # [-0.5, 0.5] (exact fp32 magic-rounding on host); on-chip
                # the embedding is a single ScalarE Sin.
                s = p * W
                t0 = raw.tile([128, W], F32, tag="t0")
                nc.sync.dma_start(out=t0, in_=rep6[:, s:s + W])
                emb = embp.tile([128, W], F16, tag="emb")
                nc.scalar.activation(emb, t0, AF.Sin, bias=zcol, scale=TWO_PI)
                nc.sync.dma_start(out=emb[36:39, :], in_=ptsh[:, s:s + W])
                nc.sync.dma_start(out=emb[100:103, :], in_=ptsh[:, s:s + W])
                return emb

            emb_next = [emb_prep(0), emb_prep(1)]

            # ---- bulk one-time weight loads ----
            bms = wp.tile_from(bmat, name="bms")  # [128, 16]
            wks = {
                i: (
                    wp.tile_from(wkh[i][0:128, :], name=f"wks{i}a"),
                    wp.tile_from(wkh[i][128:256, :], name=f"wks{i}b"),
                )
                for i in (1, 2, 3, 5, 6, 7)
            }
            w4es = wp.tile_from(w4eh, name="w4es")  # [128, 128] packed
            w4as = wp.tile_from(w4ah, name="w4as")  # [128, 256]
            w4bs = wp.tile_from(w4bh, name="w4bs")
            wsdf_a = wp.tile_from(wsdfh[0:128, :], name="wsdf_a")  # [128, 1]
            wsdf_b = wp.tile_from(wsdfh[128:256, :], name="wsdf_b")

            # h slot layout: [m0hx0, m0hx1, m1hx0, m1hx1] -> slot = 2*m + hx;
            # so a fused [128, 1024] drain per M-half gets one bias column.
            def drain(li, m, ps, h):
                dst = h[:, bass_ts(m, W)]
                bias_ap = bms[:, li * 2 + m:li * 2 + m + 1]
                if m == 0:
                    nc.scalar.activation(dst, ps, AF.Relu, bias=bias_ap)
                else:
                    nc.vector.tensor_scalar(
                        dst, ps, bias_ap, 0.0, op0=ALU.add, op1=ALU.max
                    )

            def layer(li, emb, h_prev, h3):
                h = hp.tile([128, 2 * W], F16, tag="h")
                if li == 0:
                    ps = {m: ppd.tile([128, W], F32, tag="mm2", name=f"ps0_{m}")
                          for m in (0, 1)}
                    # row-tiled K=64 pairs at array rows 0/64 run concurrent
                    for hx in (0, 1):
                        for m in (0, 1):
                            nc.tensor.matmul(
                                ps[m][:, bass_ts(hx, NT)],
                                w0s[bass_ts(m, 64), :],
                                emb[bass_ts(m, 64), bass_ts(hx, NT)],
                                start=True, stop=True,
                                tile_position=(64 * m, 0),
                            )
                    drain(li, 0, ps[0], h)
                    drain(li, 1, ps[1], h)
                    return h
                if li == 4:
                    ps = {m: ppd.tile([128, W], F32, tag="mm2", name=f"ps4_{m}")
                          for m in (0, 1)}
                    for hx in (0, 1):
                        for m in (0, 1):
                            nc.tensor.matmul(
                                ps[m][:, bass_ts(hx, NT)],
                                w4es[bass_ts(m, 64), :],
                                emb[bass_ts(m, 64), bass_ts(hx, NT)],
                                start=True, stop=False,
                                tile_position=(64 * m, 0),
                                skip_group_check=True,
                            )
                    for m in (0, 1):
                        for hx in (0, 1):
                            nc.tensor.matmul(
                                ps[m][:, bass_ts(hx, NT)],
                                w4as[:, bass_ts(m, 128)],
                                h3[:, bass_ts(hx, NT)],
                                start=False, stop=False,
                                skip_group_check=True,
                            )
                            nc.tensor.matmul(
                                ps[m][:, bass_ts(hx, NT)],
                                w4bs[:, bass_ts(m, 128)],
                                h3[:, bass_ts(2 + hx, NT)],
                                start=False, stop=True,
                                skip_group_check=True,
                            )
                        drain(li, m, ps[m], h)
                    return h
                # middle layers: K=256 as two full 128-chunks
                wa, wb = wks[li]
                for m in (0, 1):
                    ps = ppd.tile([128, W], F32, tag="mm2")
                    for hx in (0, 1):
                        nc.tensor.matmul(
                            ps[:, bass_ts(hx, NT)],
                            wa[:, bass_ts(m, 128)],
                            h_prev[:, bass_ts(hx, NT)],
                            start=True, stop=False,
                        )
                        nc.tensor.matmul(
                            ps[:, bass_ts(hx, NT)],
                            wb[:, bass_ts(m, 128)],
                            h_prev[:, bass_ts(2 + hx, NT)],
                            start=False, stop=True,
                        )
                    drain(li, m, ps, h)
                return h

            for g in range(NGROUP):
                embA, embB = emb_next
                hA = hB = None
                h3A = h3B = None
                for li in range(8):
                    hA = layer(li, embA, hA, h3A)
                    hB = layer(li, embB, hB, h3B)
                    if li == 3:
                        h3A, h3B = hA, hB
                    if li == 4 and g + 1 < NGROUP:
                        # prefetch next group's embeddings mid-group so the
                        # Sin/DMA land before this group's finals
                        emb_next = [emb_prep(2 * g + 2), emb_prep(2 * g + 3)]

                # ---- finals: 4 outputs col-packed in one PSUM bank ----
                psf = pf.tile([128, NT], F32, tag="fin")
                ofin = op_.tile([97, NT], F32, tag="ofin")
                for j, (h7, hx) in enumerate(
                    [(hA, 0), (hA, 1), (hB, 0), (hB, 1)]
                ):
                    nc.tensor.matmul(
                        psf[32 * j:32 * j + 1, :], wsdf_a,
                        h7[:, bass_ts(hx, NT)],
                        start=True, stop=False, tile_position=(0, 32 * j),
                        skip_group_check=True,
                    )
                    nc.tensor.matmul(
                        psf[32 * j:32 * j + 1, :], wsdf_b,
                        h7[:, bass_ts(2 + hx, NT)],
                        start=False, stop=True, tile_position=(0, 32 * j),
                        skip_group_check=True,
                    )
                nc.scalar.activation(
                    ofin, psf[0:97, :], AF.Identity, bias=bsdfs[0:97, 0:1]
                )
                for j in range(4):
                    nc.sync.dma_start(
                        out=out_o[4 * g + j:4 * g + j + 1, :],
                        in_=ofin[32 * j:32 * j + 1, :],
                    )
    nc.compile()
    return nc


def _prep_maps(points, ws, bs, wsdf, bsdf):
    pts = np.ascontiguousarray(points, dtype=np.float32).reshape(N, 3)
    freqs = (2.0 ** np.arange(NHARM)).astype(np.float32)
    fcol18 = (np.repeat(freqs[None, :], 3, axis=0).reshape(18, 1) / TWO_PI).astype(
        np.float32
    )

    bmat = np.zeros((128, 16), dtype=np.float32)
    for i in range(8):
        for m in range(2):
            bmat[:, i * 2 + m] = bs[i][m * 128:(m + 1) * 128]

    # packed layer-0 / layer-4-emb weights: M-half 0 at rows 0:39, M-half 1
    # at rows 64:103 (for row-tiled matmuls at array rows 0/64)
    w0p = np.zeros((128, 128), dtype=np.float16)
    w0p[0:E, :] = ws[0][:, 0:128].astype(np.float16)
    w0p[64:64 + E, :] = ws[0][:, 128:256].astype(np.float16)
    w4ep = np.zeros((128, 128), dtype=np.float16)
    w4ep[0:E, :] = ws[4][0:E, 0:128].astype(np.float16)
    w4ep[64:64 + E, :] = ws[4][0:E, 128:256].astype(np.float16)
    common = {
        "w0h": w0p,
        "w4eh": w4ep,
        "w4ah": ws[4][E:E + 128, :].astype(np.float16),
        "w4bh": ws[4][E + 128:E + 256, :].astype(np.float16),
        "wsdfh": wsdf.astype(np.float16),
        "bmat": bmat,
        "bsdf1": np.full((128, 1), float(np.ravel(bsdf)[0]), dtype=np.float32),
    }
    for i in (1, 2, 3, 5, 6, 7):
        common[f"w{i}h"] = ws[i].astype(np.float16)

    in_maps = []
    for c in range(N_CORES):
        sl = pts[c * NPC:(c + 1) * NPC]  # [NPC, 3]
        ptsT = np.ascontiguousarray(sl.T)  # [3, NPC]
        rep3 = np.repeat(ptsT, NHARM, axis=0)  # [18, NPC]
        t18 = rep3 * fcol18  # x * 2^j / (2pi), exact fp32 scaling
        # host-side range reduction to [-0.5, 0.5]: same fp32 magic-number
        # round-to-nearest the DVE trick computes, done here instead
        mg = np.float32(MAGIC)
        ys_sin = t18 - ((t18 + mg) - mg)
        tc18 = t18 + np.float32(0.25)
        ys_cos = tc18 - ((tc18 + mg) - mg)
        rep6 = np.zeros((128, NPC), dtype=np.float32)
        rep6[0:18], rep6[18:36] = ys_sin, ys_cos
        rep6[64:82], rep6[82:100] = ys_sin, ys_cos
        m = dict(common)
        m["rep6"] = rep6
        m["ptsh"] = ptsT.astype(np.float16)
        in_maps.append(m)
    return in_maps


def kernel(
    points, w0, b0, w1, b1, w2, b2, w3, b3, w4, b4, w5, b5, w6, b6, w7, b7,
    wsdf, bsdf,
):
    ws = [np.asarray(w, dtype=np.float32) for w in (w0, w1, w2, w3, w4, w5, w6, w7)]
    bs = [np.asarray(b, dtype=np.float32) for b in (b0, b1, b2, b3, b4, b5, b6, b7)]
    in_maps = _prep_maps(
        np.asarray(points), ws, bs,
        np.asarray(wsdf, dtype=np.float32), np.asarray(bsdf, dtype=np.float32),
    )

    if "nc" not in _CACHED:
        _CACHED["nc"] = _build()
    nc = _CACHED["nc"]

    res = run_bass_kernel_spmd(nc, in_maps, core_ids=list(range(N_CORES)))
    out = np.concatenate(
        [res.results[c]["out_o"] for c in range(N_CORES)], axis=0
    ).reshape(N, 1).astype(np.float32)
    return out


# revision 8
# speedup vs baseline: 1.3915x; 1.0583x over previous
"""Trainium2 Bass kernel for nn_NeuralSurface (8-layer MLP SDF with harmonic
embedding + skip concat), data-parallel over 8 NeuronCores.

Layout: activations transposed in SBUF ([feat, points]), weights stationary
fp16. Two point-pairs (A/B, 1024 pts each) interleaved per layer so the PE
never waits on ReLU drains. PSUM organized as [128, 1024] two-bank tiles per
M-half (same bias column) so each drain is a single fused ACT/DVE op. Layer-0
and layer-4 embedding chunks (K=39->64) run as row-tiled matmul pairs at array
rows 0/64 (concurrent). Harmonic sin/cos via ScalarE Sin LUT after GpSimd
range reduction (magic-number round-to-nearest). Finals col-packed 4-per-bank.
PE warmup matmuls cover the input-DMA head so HAM is at 2.4GHz from the first
real matmul.
"""

import numpy as np

import concourse.bacc as bacc
import concourse.mybir as mybir
import concourse.tile as tile
from concourse.bass_utils import run_bass_kernel_spmd

AF = mybir.ActivationFunctionType
ALU = mybir.AluOpType
F32 = mybir.dt.float32
F16 = mybir.dt.float16

N_CORES = 8
N = 262144
NPC = N // N_CORES  # 32768 points per core
NT = 512  # points per n-tile (PSUM bank / moving-operand limit)
W = 2 * NT  # pair width (1024 points)
NPAIR = NPC // W  # 32
NGROUP = NPAIR // 2  # 16 (two pairs A/B interleaved per group)
H = 256
E = 39
NHARM = 6
TWO_PI = float(2.0 * np.pi)
MAGIC = float(1.5 * 2.0**23)  # round-to-nearest via (x + M) - M
N_WARM = 26  # PE warmup matmuls (N=256) covering the input-DMA head

_CACHED = {}


def bass_ts(i, size):
    return slice(i * size, (i + 1) * size)


def _build():
    nc = bacc.Bacc("TRN2")

    rep6 = nc.dram_tensor("rep6", [128, NPC], F32, kind="ExternalInput").ap()
    ptsh = nc.dram_tensor("ptsh", [3, NPC], F16, kind="ExternalInput").ap()
    # w0/w4e packed for 64-row array tiling: rows 0:39 = M-half 0, rows
    # 64:103 = M-half 1 (matching emb duplicated at partitions 64:103).
    w0h = nc.dram_tensor("w0h", [128, 128], F16, kind="ExternalInput").ap()
    w4eh = nc.dram_tensor("w4eh", [128, 128], F16, kind="ExternalInput").ap()
    wkh = {
        i: nc.dram_tensor(f"w{i}h", [H, H], F16, kind="ExternalInput").ap()
        for i in (1, 2, 3, 5, 6, 7)
    }
    w4ah = nc.dram_tensor("w4ah", [128, H], F16, kind="ExternalInput").ap()
    w4bh = nc.dram_tensor("w4bh", [128, H], F16, kind="ExternalInput").ap()
    wsdfh = nc.dram_tensor("wsdfh", [H, 1], F16, kind="ExternalInput").ap()
    bmat = nc.dram_tensor("bmat", [128, 16], F32, kind="ExternalInput").ap()
    bsdf1 = nc.dram_tensor("bsdf1", [128, 1], F32, kind="ExternalInput").ap()
    # 2-D output (1-D ExternalOutput tensors fail NEFF load under bass2jax)
    out_o = nc.dram_tensor("out_o", [NPC // NT, NT], F32, kind="ExternalOutput").ap()

    with tile.TileContext(nc) as tc:
        with (
            tc.tile_pool(name="wp", bufs=1) as wp,
            tc.tile_pool(name="raw", bufs=3) as raw,
            tc.tile_pool(name="embp", bufs=4) as embp,
            tc.tile_pool(name="hp", bufs=6) as hp,
            tc.tile_pool(name="op", bufs=2) as op_,
            tc.tile_pool(name="ppd", bufs=3, space="PSUM") as ppd,
            tc.tile_pool(name="pf", bufs=1, space="PSUM") as pf,
        ):
            # ---- early: weights needed first + warmup ----
            w0s = wp.tile_from(w0h, name="w0s")  # [128, 128] packed
            bsdfs = wp.tile_from(bsdf1, name="bsdfs")  # [128, 1]
            zcol = wp.tile([128, 1], F32, name="zcol")
            nc.gpsimd.memset(zcol, 0.0)
            zwarm = wp.tile([128, 256], F16, name="zwarm")
            nc.gpsimd.memset(zwarm, 0.0)

            ps_warm = pf.tile([128, 256], F32, tag="warm")
            for _ in range(N_WARM):
                nc.tensor.matmul(
                    ps_warm, zwarm[:, 0:128], zwarm,
                    start=True, stop=True, skip_group_check=True,
                )

            # ---- embedding prep (t0 DMA ordered ahead of bulk weights) ----
            def emb_prep(p):
                # rep6 rows carry host-side range-reduced phases ys in
                # [-0.5, 0.5] (exact fp32 magic-rounding on host); on-chip
                # the embedding is a single ScalarE Sin.
                s = p * W
                t0 = raw.tile([128, W], F32, tag="t0")
                nc.sync.dma_start(out=t0, in_=rep6[:, s:s + W])
                emb = embp.tile([128, W], F16, tag="emb")
                nc.scalar.activation(emb, t0, AF.Sin, bias=zcol, scale=TWO_PI)
                nc.sync.dma_start(out=emb[36:39, :], in_=ptsh[:, s:s + W])
                nc.sync.dma_start(out=emb[100:103, :], in_=ptsh[:, s:s + W])
                return emb

            emb_next = [emb_prep(0), emb_prep(1)]

            # ---- bulk one-time weight loads ----
            bms = wp.tile_from(bmat, name="bms")  # [128, 16]
            wks = {
                i: (
                    wp.tile_from(wkh[i][0:128, :], name=f"wks{i}a"),
                    wp.tile_from(wkh[i][128:256, :], name=f"wks{i}b"),
                )
                for i in (1, 2, 3, 5, 6, 7)
            }
            w4es = wp.tile_from(w4eh, name="w4es")  # [128, 128] packed
            w4as = wp.tile_from(w4ah, name="w4as")  # [128, 256]
            w4bs = wp.tile_from(w4bh, name="w4bs")
            wsdf_a = wp.tile_from(wsdfh[0:128, :], name="wsdf_a")  # [128, 1]
            wsdf_b = wp.tile_from(wsdfh[128:256, :], name="wsdf_b")

            # h slot layout: [m0hx0, m0hx1, m1hx0, m1hx1] -> slot = 2*m + hx;
            # so a fused [128, 1024] drain per M-half gets one bias column.
            def drain(li, m, ps, h):
                dst = h[:, bass_ts(m, W)]
                bias_ap = bms[:, li * 2 + m:li * 2 + m + 1]
                if m == 0:
                    nc.scalar.activation(dst, ps, AF.Relu, bias=bias_ap)
                else:
                    nc.vector.tensor_scalar(
                        dst, ps, bias_ap, 0.0, op0=ALU.add, op1=ALU.max
                    )

            def layer(li, emb, h_prev, h3):
                h = hp.tile([128, 2 * W], F16, tag="h")
                if li == 0:
                    ps = {m: ppd.tile([128, W], F32, tag="mm2", name=f"ps0_{m}")
                          for m in (0, 1)}
                    # row-tiled K=64 pairs at array rows 0/64 run concurrent
                    for hx in (0, 1):
                        for m in (0, 1):
                            nc.tensor.matmul(
                                ps[m][:, bass_ts(hx, NT)],
                                w0s[bass_ts(m, 64), :],
                                emb[bass_ts(m, 64), bass_ts(hx, NT)],
                                start=True, stop=True,
                                tile_position=(64 * m, 0),
                            )
                    drain(li, 0, ps[0], h)
                    drain(li, 1, ps[1], h)
                    return h
                if li == 4:
                    ps = {m: ppd.tile([128, W], F32, tag="mm2", name=f"ps4_{m}")
                          for m in (0, 1)}
                    for hx in (0, 1):
                        for m in (0, 1):
                            nc.tensor.matmul(
                                ps[m][:, bass_ts(hx, NT)],
                                w4es[bass_ts(m, 64), :],
                                emb[bass_ts(m, 64), bass_ts(hx, NT)],
                                start=True, stop=False,
                                tile_position=(64 * m, 0),
                                skip_group_check=True,
                            )
                    for m in (0, 1):
                        for hx in (0, 1):
                            nc.tensor.matmul(
                                ps[m][:, bass_ts(hx, NT)],
                                w4as[:, bass_ts(m, 128)],
                                h3[:, bass_ts(hx, NT)],
                                start=False, stop=False,
                                skip_group_check=True,
                            )
                            nc.tensor.matmul(
                                ps[m][:, bass_ts(hx, NT)],
                                w4bs[:, bass_ts(m, 128)],
                                h3[:, bass_ts(2 + hx, NT)],
                                start=False, stop=True,
                                skip_group_check=True,
                            )
                        drain(li, m, ps[m], h)
                    return h
                # middle layers: K=256 as two full 128-chunks
                wa, wb = wks[li]
                for m in (0, 1):
                    ps = ppd.tile([128, W], F32, tag="mm2")
                    for hx in (0, 1):
                        nc.tensor.matmul(
                            ps[:, bass_ts(hx, NT)],
                            wa[:, bass_ts(m, 128)],
                            h_prev[:, bass_ts(hx, NT)],
                            start=True, stop=False,
                        )
                        nc.tensor.matmul(
                            ps[:, bass_ts(hx, NT)],
                            wb[:, bass_ts(m, 128)],
                            h_prev[:, bass_ts(2 + hx, NT)],
                            start=False, stop=True,
                        )
                    drain(li, m, ps, h)
                return h

            # finals: 4 outputs col-packed in one PSUM bank; emitted AFTER
            # the next group's L0 so ~2us of independent PE work covers the
            # previous group's L7 drain tail (no PE stall on ReLU/PSUM WAR)
            def finals(g, hA, hB):
                psf = pf.tile([128, NT], F32, tag="fin")
                ofin = op_.tile([97, NT], F32, tag="ofin")
                for j, (h7, hx) in enumerate(
                    [(hA, 0), (hA, 1), (hB, 0), (hB, 1)]
                ):
                    nc.tensor.matmul(
                        psf[32 * j:32 * j + 1, :], wsdf_a,
                        h7[:, bass_ts(hx, NT)],
                        start=True, stop=False, tile_position=(0, 32 * j),
                        skip_group_check=True,
                    )
                    nc.tensor.matmul(
                        psf[32 * j:32 * j + 1, :], wsdf_b,
                        h7[:, bass_ts(2 + hx, NT)],
                        start=False, stop=True, tile_position=(0, 32 * j),
                        skip_group_check=True,
                    )
                nc.scalar.activation(
                    ofin, psf[0:97, :], AF.Identity, bias=bsdfs[0:97, 0:1]
                )
                for j in range(4):
                    nc.sync.dma_start(
                        out=out_o[4 * g + j:4 * g + j + 1, :],
                        in_=ofin[32 * j:32 * j + 1, :],
                    )

            pending = None  # (g, h7A, h7B) awaiting finals
            for g in range(NGROUP):
                embA, embB = emb_next
                hA = hB = None
                h3A = h3B = None
                for li in range(8):
                    hA = layer(li, embA, hA, h3A)
                    hB = layer(li, embB, hB, h3B)
                    if li == 0 and pending is not None:
                        finals(*pending)
                        pending = None
                    if li == 3:
                        h3A, h3B = hA, hB
                    if li == 4 and g + 1 < NGROUP:
                        # prefetch next group's embeddings mid-group
                        emb_next = [emb_prep(2 * g + 2), emb_prep(2 * g + 3)]
                pending = (g, hA, hB)
            finals(*pending)
    nc.compile()
    return nc


def _prep_maps(points, ws, bs, wsdf, bsdf):
    pts = np.ascontiguousarray(points, dtype=np.float32).reshape(N, 3)
    freqs = (2.0 ** np.arange(NHARM)).astype(np.float32)
    fcol18 = (np.repeat(freqs[None, :], 3, axis=0).reshape(18, 1) / TWO_PI).astype(
        np.float32
    )

    bmat = np.zeros((128, 16), dtype=np.float32)
    for i in range(8):
        for m in range(2):
            bmat[:, i * 2 + m] = bs[i][m * 128:(m + 1) * 128]

    # packed layer-0 / layer-4-emb weights: M-half 0 at rows 0:39, M-half 1
    # at rows 64:103 (for row-tiled matmuls at array rows 0/64)
    w0p = np.zeros((128, 128), dtype=np.float16)
    w0p[0:E, :] = ws[0][:, 0:128].astype(np.float16)
    w0p[64:64 + E, :] = ws[0][:, 128:256].astype(np.float16)
    w4ep = np.zeros((128, 128), dtype=np.float16)
    w4ep[0:E, :] = ws[4][0:E, 0:128].astype(np.float16)
    w4ep[64:64 + E, :] = ws[4][0:E, 128:256].astype(np.float16)
    common = {
        "w0h": w0p,
        "w4eh": w4ep,
        "w4ah": ws[4][E:E + 128, :].astype(np.float16),
        "w4bh": ws[4][E + 128:E + 256, :].astype(np.float16),
        "wsdfh": wsdf.astype(np.float16),
        "bmat": bmat,
        "bsdf1": np.full((128, 1), float(np.ravel(bsdf)[0]), dtype=np.float32),
    }
    for i in (1, 2, 3, 5, 6, 7):
        common[f"w{i}h"] = ws[i].astype(np.float16)

    in_maps = []
    for c in range(N_CORES):
        sl = pts[c * NPC:(c + 1) * NPC]  # [NPC, 3]
        ptsT = np.ascontiguousarray(sl.T)  # [3, NPC]
        rep3 = np.repeat(ptsT, NHARM, axis=0)  # [18, NPC]
        t18 = rep3 * fcol18  # x * 2^j / (2pi), exact fp32 scaling
        # host-side range reduction to [-0.5, 0.5]: same fp32 magic-number
        # round-to-nearest the DVE trick computes, done here instead
        mg = np.float32(MAGIC)
        ys_sin = t18 - ((t18 + mg) - mg)
        tc18 = t18 + np.float32(0.25)
        ys_cos = tc18 - ((tc18 + mg) - mg)
        rep6 = np.zeros((128, NPC), dtype=np.float32)
        rep6[0:18], rep6[18:36] = ys_sin, ys_cos
        rep6[64:82], rep6[82:100] = ys_sin, ys_cos
        m = dict(common)
        m["rep6"] = rep6
        m["ptsh"] = ptsT.astype(np.float16)
        in_maps.append(m)
    return in_maps


def kernel(
    points, w0, b0, w1, b1, w2, b2, w3, b3, w4, b4, w5, b5, w6, b6, w7, b7,
    wsdf, bsdf,
):
    ws = [np.asarray(w, dtype=np.float32) for w in (w0, w1, w2, w3, w4, w5, w6, w7)]
    bs = [np.asarray(b, dtype=np.float32) for b in (b0, b1, b2, b3, b4, b5, b6, b7)]
    in_maps = _prep_maps(
        np.asarray(points), ws, bs,
        np.asarray(wsdf, dtype=np.float32), np.asarray(bsdf, dtype=np.float32),
    )

    if "nc" not in _CACHED:
        _CACHED["nc"] = _build()
    nc = _CACHED["nc"]

    res = run_bass_kernel_spmd(nc, in_maps, core_ids=list(range(N_CORES)))
    out = np.concatenate(
        [res.results[c]["out_o"] for c in range(N_CORES)], axis=0
    ).reshape(N, 1).astype(np.float32)
    return out


# revision 9
# speedup vs baseline: 1.4469x; 1.0398x over previous
"""Trainium2 Bass kernel for nn_NeuralSurface (8-layer MLP SDF with harmonic
embedding + skip concat), data-parallel over 8 NeuronCores.

Layout: activations transposed in SBUF ([feat, points]), weights stationary
fp16. Two point-pairs (A/B, 1024 pts each) interleaved per layer so the PE
never waits on ReLU drains. PSUM as 7 rotating single-bank [128,512] tiles;
each bank drains (fused bias+ReLU, ACT for M-half 0 / DVE for M-half 1)
right after its two accumulating matmuls, maximizing WAR slack. Layer-0 and
layer-4 embedding chunks (K=39->64) run as row-tiled matmul pairs at array
rows 0/64 (concurrent). Finals col-packed 4-per-bank (concurrent), deferred
past the next group's L0 so the PE never waits on the L7 drain tail.
Embedding phases are range-reduced host-side; on-chip it is a single Sin.
ptsh/output DMAs issue from the idle GpSimd queue to keep the Sync DMA FIFO
from head-of-line blocking. PE warmup matmuls cover the input-DMA head so
HAM is at 2.4GHz from the first real matmul.
"""

import numpy as np

import concourse.bacc as bacc
import concourse.mybir as mybir
import concourse.tile as tile
from concourse.bass_utils import run_bass_kernel_spmd

AF = mybir.ActivationFunctionType
ALU = mybir.AluOpType
F32 = mybir.dt.float32
F16 = mybir.dt.float16

N_CORES = 8
N = 262144
NPC = N // N_CORES  # 32768 points per core
NT = 512  # points per n-tile (PSUM bank / moving-operand limit)
W = 2 * NT  # pair width (1024 points)
NPAIR = NPC // W  # 32
NGROUP = NPAIR // 2  # 16 (two pairs A/B interleaved per group)
H = 256
E = 39
NHARM = 6
TWO_PI = float(2.0 * np.pi)
MAGIC = float(1.5 * 2.0**23)  # round-to-nearest via (x + M) - M
N_WARM = 30  # PE warmup matmuls (N=256) covering the input-DMA head

_CACHED = {}


def bass_ts(i, size):
    return slice(i * size, (i + 1) * size)


def _build():
    nc = bacc.Bacc("TRN2")

    rep6 = nc.dram_tensor("rep6", [128, NPC], F32, kind="ExternalInput").ap()
    ptsh = nc.dram_tensor("ptsh", [3, NPC], F16, kind="ExternalInput").ap()
    # w0/w4e packed for 64-row array tiling: rows 0:39 = M-half 0, rows
    # 64:103 = M-half 1 (matching emb duplicated at partitions 64:103).
    w0h = nc.dram_tensor("w0h", [128, 128], F16, kind="ExternalInput").ap()
    w4eh = nc.dram_tensor("w4eh", [128, 128], F16, kind="ExternalInput").ap()
    wkh = {
        i: nc.dram_tensor(f"w{i}h", [H, H], F16, kind="ExternalInput").ap()
        for i in (1, 2, 3, 5, 6, 7)
    }
    w4ah = nc.dram_tensor("w4ah", [128, H], F16, kind="ExternalInput").ap()
    w4bh = nc.dram_tensor("w4bh", [128, H], F16, kind="ExternalInput").ap()
    wsdfh = nc.dram_tensor("wsdfh", [H, 1], F16, kind="ExternalInput").ap()
    bmat = nc.dram_tensor("bmat", [128, 16], F32, kind="ExternalInput").ap()
    bsdf1 = nc.dram_tensor("bsdf1", [128, 1], F32, kind="ExternalInput").ap()
    # 2-D output (1-D ExternalOutput tensors fail NEFF load under bass2jax)
    out_o = nc.dram_tensor("out_o", [NPC // NT, NT], F32, kind="ExternalOutput").ap()

    with tile.TileContext(nc) as tc:
        with (
            tc.tile_pool(name="wp", bufs=1) as wp,
            tc.tile_pool(name="raw", bufs=3) as raw,
            tc.tile_pool(name="embp", bufs=4) as embp,
            tc.tile_pool(name="hp", bufs=6) as hp,
            tc.tile_pool(name="op", bufs=2) as op_,
            tc.tile_pool(name="ppd", bufs=7, space="PSUM") as ppd,
            tc.tile_pool(name="pf", bufs=1, space="PSUM") as pf,
        ):
            # ---- head: hoist the Sin LUT (table DMA + ACT_TABLE_LOAD)
            # ahead of the bulk input DMAs via a dummy 1-element Sin ----
            zcol = wp.tile([128, 1], F32, name="zcol")
            nc.gpsimd.memset(zcol, 0.0)
            zsin = wp.tile([1, 1], F16, name="zsin")
            nc.scalar.activation(zsin, zcol[0:1, 0:1], AF.Sin, bias=zcol[0:1, 0:1])

            w0s = wp.tile_from(w0h, name="w0s")  # [128, 128] packed
            bsdfs = wp.tile_from(bsdf1, name="bsdfs")  # [128, 1]
            zwarm = wp.tile([128, 256], F16, name="zwarm")
            nc.gpsimd.memset(zwarm, 0.0)

            ps_warm = ppd.tile([128, NT], F32, tag="mm", name="ps_warm")
            for _ in range(N_WARM):
                nc.tensor.matmul(
                    ps_warm[:, 0:256], zwarm[:, 0:128], zwarm,
                    start=True, stop=True, skip_group_check=True,
                )

            # ---- embedding prep (t0 DMA on sync; ptsh on gpsimd queue) ----
            def emb_prep(p):
                # rep6 rows carry host-side range-reduced phases in
                # [-0.5, 0.5]; on-chip the embedding is a single Sin.
                s = p * W
                t0 = raw.tile([128, W], F32, tag="t0")
                nc.sync.dma_start(out=t0, in_=rep6[:, s:s + W])
                emb = embp.tile([128, W], F16, tag="emb")
                nc.scalar.activation(emb, t0, AF.Sin, bias=zcol, scale=TWO_PI)
                nc.gpsimd.dma_start(out=emb[36:39, :], in_=ptsh[:, s:s + W])
                nc.gpsimd.dma_start(out=emb[100:103, :], in_=ptsh[:, s:s + W])
                return emb

            emb_next = [emb_prep(0), emb_prep(1)]

            # ---- bulk one-time weight loads ----
            bms = wp.tile_from(bmat, name="bms")  # [128, 16]
            wks = {
                i: (
                    wp.tile_from(wkh[i][0:128, :], name=f"wks{i}a"),
                    wp.tile_from(wkh[i][128:256, :], name=f"wks{i}b"),
                )
                for i in (1, 2, 3, 5, 6, 7)
            }
            w4es = wp.tile_from(w4eh, name="w4es")  # [128, 128] packed
            w4as = wp.tile_from(w4ah, name="w4as")  # [128, 256]
            w4bs = wp.tile_from(w4bh, name="w4bs")
            wsdf_a = wp.tile_from(wsdfh[0:128, :], name="wsdf_a")  # [128, 1]
            wsdf_b = wp.tile_from(wsdfh[128:256, :], name="wsdf_b")

            # h slot layout: [m0hx0, m0hx1, m1hx0, m1hx1] -> slot = 2*m + hx
            def drain(li, m, hx, ps, h):
                dst = h[:, bass_ts(2 * m + hx, NT)]
                bias_ap = bms[:, li * 2 + m:li * 2 + m + 1]
                if m == 0:
                    nc.scalar.activation(dst, ps, AF.Relu, bias=bias_ap)
                else:
                    nc.vector.tensor_scalar(
                        dst, ps, bias_ap, 0.0, op0=ALU.add, op1=ALU.max
                    )

            def layer(li, emb, h_prev, h3):
                h = hp.tile([128, 2 * W], F16, tag="h")
                if li == 0:
                    # row-tiled K=64 pairs at array rows 0/64 run concurrent
                    for hx in (0, 1):
                        for m in (0, 1):
                            ps = ppd.tile([128, NT], F32, tag="mm",
                                          name=f"ps0_{m}{hx}")
                            nc.tensor.matmul(
                                ps, w0s[bass_ts(m, 64), :],
                                emb[bass_ts(m, 64), bass_ts(hx, NT)],
                                start=True, stop=True,
                                tile_position=(64 * m, 0),
                            )
                            drain(li, m, hx, ps, h)
                    return h
                if li == 4:
                    ps = {}
                    for hx in (0, 1):
                        for m in (0, 1):
                            ps[(m, hx)] = ppd.tile([128, NT], F32, tag="mm",
                                                   name=f"ps4_{m}{hx}")
                            nc.tensor.matmul(
                                ps[(m, hx)], w4es[bass_ts(m, 64), :],
                                emb[bass_ts(m, 64), bass_ts(hx, NT)],
                                start=True, stop=False,
                                tile_position=(64 * m, 0),
                                skip_group_check=True,
                            )
                    for m in (0, 1):
                        for hx in (0, 1):
                            nc.tensor.matmul(
                                ps[(m, hx)], w4as[:, bass_ts(m, 128)],
                                h3[:, bass_ts(hx, NT)],
                                start=False, stop=False,
                                skip_group_check=True,
                            )
                            nc.tensor.matmul(
                                ps[(m, hx)], w4bs[:, bass_ts(m, 128)],
                                h3[:, bass_ts(2 + hx, NT)],
                                start=False, stop=True,
                                skip_group_check=True,
                            )
                            drain(li, m, hx, ps[(m, hx)], h)
                    return h
                # middle layers: K=256 as two full 128-chunks per bank
                wa, wb = wks[li]
                for m in (0, 1):
                    for hx in (0, 1):
                        ps = ppd.tile([128, NT], F32, tag="mm",
                                      name=f"psm_{m}{hx}")
                        nc.tensor.matmul(
                            ps, wa[:, bass_ts(m, 128)],
                            h_prev[:, bass_ts(hx, NT)],
                            start=True, stop=False,
                        )
                        nc.tensor.matmul(
                            ps, wb[:, bass_ts(m, 128)],
                            h_prev[:, bass_ts(2 + hx, NT)],
                            start=False, stop=True,
                        )
                        drain(li, m, hx, ps, h)
                return h

            # finals: 4 outputs col-packed in one PSUM bank (concurrent);
            # emitted AFTER the next group's L0 so ~2us of independent PE
            # work covers the previous group's L7 drain tail
            def finals(g, hA, hB):
                psf = pf.tile([128, NT], F32, tag="fin")
                ofin = op_.tile([97, NT], F32, tag="ofin")
                for j, (h7, hx) in enumerate(
                    [(hA, 0), (hA, 1), (hB, 0), (hB, 1)]
                ):
                    nc.tensor.matmul(
                        psf[32 * j:32 * j + 1, :], wsdf_a,
                        h7[:, bass_ts(hx, NT)],
                        start=True, stop=False, tile_position=(0, 32 * j),
                        skip_group_check=True,
                    )
                    nc.tensor.matmul(
                        psf[32 * j:32 * j + 1, :], wsdf_b,
                        h7[:, bass_ts(2 + hx, NT)],
                        start=False, stop=True, tile_position=(0, 32 * j),
                        skip_group_check=True,
                    )
                nc.scalar.activation(
                    ofin, psf[0:97, :], AF.Identity, bias=bsdfs[0:97, 0:1]
                )
                for j in range(4):
                    nc.gpsimd.dma_start(
                        out=out_o[4 * g + j:4 * g + j + 1, :],
                        in_=ofin[32 * j:32 * j + 1, :],
                    )

            pending = None  # (g, h7A, h7B) awaiting finals
            for g in range(NGROUP):
                embA, embB = emb_next
                nextA = nextB = None
                hA = hB = None
                h3A = h3B = None
                for li in range(8):
                    hA = layer(li, embA, hA, h3A)
                    hB = layer(li, embB, hB, h3B)
                    if li == 0 and pending is not None:
                        finals(*pending)
                        pending = None
                    if li == 3:
                        h3A, h3B = hA, hB
                        if g + 1 < NGROUP:
                            nextA = emb_prep(2 * g + 2)
                    if li == 5 and g + 1 < NGROUP:
                        nextB = emb_prep(2 * g + 3)
                pending = (g, hA, hB)
                emb_next = [nextA, nextB]
            finals(*pending)
    nc.compile()
    return nc


def _prep_maps(points, ws, bs, wsdf, bsdf):
    pts = np.ascontiguousarray(points, dtype=np.float32).reshape(N, 3)
    freqs = (2.0 ** np.arange(NHARM)).astype(np.float32)
    fcol18 = (np.repeat(freqs[None, :], 3, axis=0).reshape(18, 1) / TWO_PI).astype(
        np.float32
    )

    bmat = np.zeros((128, 16), dtype=np.float32)
    for i in range(8):
        for m in range(2):
            bmat[:, i * 2 + m] = bs[i][m * 128:(m + 1) * 128]

    # packed layer-0 / layer-4-emb weights: M-half 0 at rows 0:39, M-half 1
    # at rows 64:103 (for row-tiled matmuls at array rows 0/64)
    w0p = np.zeros((128, 128), dtype=np.float16)
    w0p[0:E, :] = ws[0][:, 0:128].astype(np.float16)
    w0p[64:64 + E, :] = ws[0][:, 128:256].astype(np.float16)
    w4ep = np.zeros((128, 128), dtype=np.float16)
    w4ep[0:E, :] = ws[4][0:E, 0:128].astype(np.float16)
    w4ep[64:64 + E, :] = ws[4][0:E, 128:256].astype(np.float16)
    common = {
        "w0h": w0p,
        "w4eh": w4ep,
        "w4ah": ws[4][E:E + 128, :].astype(np.float16),
        "w4bh": ws[4][E + 128:E + 256, :].astype(np.float16),
        "wsdfh": wsdf.astype(np.float16),
        "bmat": bmat,
        "bsdf1": np.full((128, 1), float(np.ravel(bsdf)[0]), dtype=np.float32),
    }
    for i in (1, 2, 3, 5, 6, 7):
        common[f"w{i}h"] = ws[i].astype(np.float16)

    in_maps = []
    for c in range(N_CORES):
        sl = pts[c * NPC:(c + 1) * NPC]  # [NPC, 3]
        ptsT = np.ascontiguousarray(sl.T)  # [3, NPC]
        rep3 = np.repeat(ptsT, NHARM, axis=0)  # [18, NPC]
        t18 = rep3 * fcol18  # x * 2^j / (2pi), exact fp32 scaling
        # host-side range reduction to [-0.5, 0.5]: fp32 magic-number
        # round-to-nearest, same arithmetic the DVE trick would do
        mg = np.float32(MAGIC)
        ys_sin = t18 - ((t18 + mg) - mg)
        tc18 = t18 + np.float32(0.25)
        ys_cos = tc18 - ((tc18 + mg) - mg)
        rep6 = np.zeros((128, NPC), dtype=np.float32)
        rep6[0:18], rep6[18:36] = ys_sin, ys_cos
        rep6[64:82], rep6[82:100] = ys_sin, ys_cos
        m = dict(common)
        m["rep6"] = rep6
        m["ptsh"] = ptsT.astype(np.float16)
        in_maps.append(m)
    return in_maps


def kernel(
    points, w0, b0, w1, b1, w2, b2, w3, b3, w4, b4, w5, b5, w6, b6, w7, b7,
    wsdf, bsdf,
):
    ws = [np.asarray(w, dtype=np.float32) for w in (w0, w1, w2, w3, w4, w5, w6, w7)]
    bs = [np.asarray(b, dtype=np.float32) for b in (b0, b1, b2, b3, b4, b5, b6, b7)]
    in_maps = _prep_maps(
        np.asarray(points), ws, bs,
        np.asarray(wsdf, dtype=np.float32), np.asarray(bsdf, dtype=np.float32),
    )

    if "nc" not in _CACHED:
        _CACHED["nc"] = _build()
    nc = _CACHED["nc"]

    res = run_bass_kernel_spmd(nc, in_maps, core_ids=list(range(N_CORES)))
    out = np.concatenate(
        [res.results[c]["out_o"] for c in range(N_CORES)], axis=0
    ).reshape(N, 1).astype(np.float32)
    return out


# revision 17
# speedup vs baseline: 1.4774x; 1.0211x over previous
"""Trainium2 Bass kernel for nn_NeuralSurface (8-layer MLP SDF with harmonic
embedding + skip concat), data-parallel over 8 NeuronCores.

Layout: activations transposed in SBUF ([feat, points]), weights stationary
fp16. Two point-pairs (A/B, 1024 pts each) interleaved per layer so the PE
never waits on ReLU drains. PSUM as 7 rotating single-bank [128,512] tiles;
each bank drains (fused bias+ReLU, ACT for M-half 0 / DVE for M-half 1)
right after its two accumulating matmuls, maximizing WAR slack. Layer-0 and
layer-4 embedding chunks (K=39->64) run as row-tiled matmul pairs at array
rows 0/64 (concurrent). Finals col-packed 4-per-bank (concurrent), deferred
past the next group's L0 so the PE never waits on the L7 drain tail.
Embedding phases are range-reduced host-side; on-chip it is a single Sin.
ptsh/output DMAs issue from the idle GpSimd queue to keep the Sync DMA FIFO
from head-of-line blocking. PE warmup matmuls cover the input-DMA head so
HAM is at 2.4GHz from the first real matmul.
"""

import numpy as np

import concourse.bacc as bacc
import concourse.mybir as mybir
import concourse.tile as tile
from concourse.bass_utils import run_bass_kernel_spmd

AF = mybir.ActivationFunctionType
ALU = mybir.AluOpType
F32 = mybir.dt.float32
F16 = mybir.dt.float16

N_CORES = 8
N = 262144
NPC = N // N_CORES  # 32768 points per core
NT = 512  # points per n-tile (PSUM bank / moving-operand limit)
W = 2 * NT  # pair width (1024 points)
NPAIR = NPC // W  # 32
NGROUP = NPAIR // 2  # 16 (two pairs A/B interleaved per group)
H = 256
E = 39
NHARM = 6
TWO_PI = float(2.0 * np.pi)
MAGIC = float(1.5 * 2.0**23)  # round-to-nearest via (x + M) - M
N_WARM = 30  # PE warmup matmuls (N=256) covering the input-DMA head

_CACHED = {}


def bass_ts(i, size):
    return slice(i * size, (i + 1) * size)


def _build():
    nc = bacc.Bacc("TRN2")

    rep6 = nc.dram_tensor("rep6", [128, NPC], F32, kind="ExternalInput").ap()
    ptsh = nc.dram_tensor("ptsh", [3, NPC], F16, kind="ExternalInput").ap()
    # all fp16 weights consolidated into one [128, 3842] tensor -> ONE DMA:
    # w0p(128) | w4ep(128) | w{1,2,3,5,6,7}{a,b}(12x256) | w4a(256) |
    # w4b(256) | wsdf_a(1) | wsdf_b(1).  w0p/w4ep packed for 64-row array
    # tiling: rows 0:39 = M-half 0, rows 64:103 = M-half 1.
    WALL = 128 + 128 + 12 * 256 + 512 + 2  # 3842
    wallh = nc.dram_tensor("wallh", [128, WALL], F16, kind="ExternalInput").ap()
    bmat = nc.dram_tensor("bmat", [128, 16], F32, kind="ExternalInput").ap()
    bsdf1 = nc.dram_tensor("bsdf1", [128, 1], F32, kind="ExternalInput").ap()
    # 2-D output (1-D ExternalOutput tensors fail NEFF load under bass2jax)
    out_o = nc.dram_tensor("out_o", [NPC // NT, NT], F32, kind="ExternalOutput").ap()

    with tile.TileContext(nc) as tc:
        with (
            tc.tile_pool(name="wp", bufs=1) as wp,
            tc.tile_pool(name="raw", bufs=3) as raw,
            tc.tile_pool(name="embp", bufs=4) as embp,
            tc.tile_pool(name="hp", bufs=6) as hp,
            tc.tile_pool(name="op", bufs=2) as op_,
            tc.tile_pool(name="ppd", bufs=7, space="PSUM") as ppd,
            tc.tile_pool(name="pf", bufs=1, space="PSUM") as pf,
        ):
            # ---- head: hoist the Sin LUT (table DMA + ACT_TABLE_LOAD)
            # ahead of the bulk input DMAs via a dummy 1-element Sin ----
            zcol = wp.tile([128, 1], F32, name="zcol")
            nc.gpsimd.memset(zcol, 0.0)

            # ---- bulk one-time weight load: ONE DMA on the scalar queue so
            # its transfer parallels the t0 transfers on sync/vector ----
            wall = wp.tile([128, WALL], F16, name="wall")
            nc.scalar.dma_start(out=wall, in_=wallh)
            zwarm = wp.tile([128, 256], F16, name="zwarm")
            nc.gpsimd.memset(zwarm, 0.0)

            w0s = wall[:, 0:128]
            w4es = wall[:, 128:256]
            wks = {}
            off = 256
            for i in (1, 2, 3, 5, 6, 7):
                wks[i] = (wall[:, off:off + 256], wall[:, off + 256:off + 512])
                off += 512
            w4as = wall[:, off:off + 256]
            w4bs = wall[:, off + 256:off + 512]
            wsdf_a = wall[:, off + 512:off + 513]
            wsdf_b = wall[:, off + 513:off + 514]

            ps_warm = ppd.tile([128, NT], F32, tag="mm", name="ps_warm")
            for _ in range(N_WARM):
                nc.tensor.matmul(
                    ps_warm[:, 0:256], zwarm[:, 0:128], zwarm,
                    start=True, stop=True, skip_group_check=True,
                )

            # ---- embedding prep; pair 1's t0 rides the scalar queue so the
            # two head t0 transfers run in parallel ----
            def emb_prep(p):
                # rep6 rows carry host-side range-reduced phases in
                # [-0.5, 0.5]; on-chip the embedding is a single Sin.
                s = p * W
                eng = nc.scalar if p == 1 else nc.sync
                t0 = raw.tile([128, W], F32, tag="t0")
                eng.dma_start(out=t0, in_=rep6[:, s:s + W])
                emb = embp.tile([128, W], F16, tag="emb")
                nc.scalar.activation(emb, t0, AF.Sin, bias=zcol, scale=TWO_PI)
                nc.sync.dma_start(out=emb[36:39, :], in_=ptsh[:, s:s + W])
                nc.sync.dma_start(out=emb[100:103, :], in_=ptsh[:, s:s + W])
                return emb

            emb_next = [emb_prep(0), emb_prep(1)]
            bsdfs = wp.tile_from(bsdf1, name="bsdfs")  # [128, 1]
            bms = wp.tile_from(bmat, name="bms")  # [128, 16]

            # h slot layout: [m0hx0, m0hx1, m1hx0, m1hx1] -> slot = 2*m + hx
            def drain(li, m, hx, ps, h):
                dst = h[:, bass_ts(2 * m + hx, NT)]
                bias_ap = bms[:, li * 2 + m:li * 2 + m + 1]
                if m == 0:
                    nc.scalar.activation(dst, ps, AF.Relu, bias=bias_ap)
                else:
                    nc.vector.tensor_scalar(
                        dst, ps, bias_ap, 0.0, op0=ALU.add, op1=ALU.max
                    )

            def layer(li, emb, h_prev, h3):
                h = hp.tile([128, 2 * W], F16, tag="h")
                if li == 0:
                    # row-tiled K=64 pairs at array rows 0/64 run concurrent
                    for hx in (0, 1):
                        for m in (0, 1):
                            ps = ppd.tile([128, NT], F32, tag="mm",
                                          name=f"ps0_{m}{hx}")
                            nc.tensor.matmul(
                                ps, w0s[bass_ts(m, 64), :],
                                emb[bass_ts(m, 64), bass_ts(hx, NT)],
                                start=True, stop=True,
                                tile_position=(64 * m, 0),
                            )
                            drain(li, m, hx, ps, h)
                    return h
                if li == 4:
                    ps = {}
                    for hx in (0, 1):
                        for m in (0, 1):
                            ps[(m, hx)] = ppd.tile([128, NT], F32, tag="mm",
                                                   name=f"ps4_{m}{hx}")
                            nc.tensor.matmul(
                                ps[(m, hx)], w4es[bass_ts(m, 64), :],
                                emb[bass_ts(m, 64), bass_ts(hx, NT)],
                                start=True, stop=False,
                                tile_position=(64 * m, 0),
                                skip_group_check=True,
                            )
                    for m in (0, 1):
                        for hx in (0, 1):
                            nc.tensor.matmul(
                                ps[(m, hx)], w4as[:, bass_ts(m, 128)],
                                h3[:, bass_ts(hx, NT)],
                                start=False, stop=False,
                                skip_group_check=True,
                            )
                            nc.tensor.matmul(
                                ps[(m, hx)], w4bs[:, bass_ts(m, 128)],
                                h3[:, bass_ts(2 + hx, NT)],
                                start=False, stop=True,
                                skip_group_check=True,
                            )
                            drain(li, m, hx, ps[(m, hx)], h)
                    return h
                # middle layers: K=256 as two full 128-chunks per bank
                wa, wb = wks[li]
                for m in (0, 1):
                    for hx in (0, 1):
                        ps = ppd.tile([128, NT], F32, tag="mm",
                                      name=f"psm_{m}{hx}")
                        nc.tensor.matmul(
                            ps, wa[:, bass_ts(m, 128)],
                            h_prev[:, bass_ts(hx, NT)],
                            start=True, stop=False,
                        )
                        nc.tensor.matmul(
                            ps, wb[:, bass_ts(m, 128)],
                            h_prev[:, bass_ts(2 + hx, NT)],
                            start=False, stop=True,
                        )
                        drain(li, m, hx, ps, h)
                return h

            # finals: 4 outputs col-packed in one PSUM bank (concurrent);
            # emitted AFTER the next group's L0 so ~2us of independent PE
            # work covers the previous group's L7 drain tail
            def finals(g, hA, hB):
                psf = pf.tile([128, NT], F32, tag="fin")
                ofin = op_.tile([97, NT], F32, tag="ofin")
                for j, (h7, hx) in enumerate(
                    [(hA, 0), (hA, 1), (hB, 0), (hB, 1)]
                ):
                    nc.tensor.matmul(
                        psf[32 * j:32 * j + 1, :], wsdf_a,
                        h7[:, bass_ts(hx, NT)],
                        start=True, stop=False, tile_position=(0, 32 * j),
                        skip_group_check=True,
                    )
                    nc.tensor.matmul(
                        psf[32 * j:32 * j + 1, :], wsdf_b,
                        h7[:, bass_ts(2 + hx, NT)],
                        start=False, stop=True, tile_position=(0, 32 * j),
                        skip_group_check=True,
                    )
                nc.scalar.activation(
                    ofin, psf[0:97, :], AF.Identity, bias=bsdfs[0:97, 0:1]
                )
                for j in range(4):
                    nc.sync.dma_start(
                        out=out_o[4 * g + j:4 * g + j + 1, :],
                        in_=ofin[32 * j:32 * j + 1, :],
                    )

            pending = None  # (g, h7A, h7B) awaiting finals
            for g in range(NGROUP):
                embA, embB = emb_next
                nextA = nextB = None
                hA = hB = None
                h3A = h3B = None
                for li in range(8):
                    hA = layer(li, embA, hA, h3A)
                    if li == 0 and pending is not None:
                        # between L0A and L0B: finals' 8 matmuls use no ppd
                        # banks, giving L0B's PSUM WAR extra slack
                        finals(*pending)
                        pending = None
                    hB = layer(li, embB, hB, h3B)
                    if li == 3:
                        h3A, h3B = hA, hB
                        if g + 1 < NGROUP:
                            nextA = emb_prep(2 * g + 2)
                    if li == 5 and g + 1 < NGROUP:
                        nextB = emb_prep(2 * g + 3)
                pending = (g, hA, hB)
                emb_next = [nextA, nextB]
            finals(*pending)
    nc.compile()
    return nc


def _prep_maps(points, ws, bs, wsdf, bsdf):
    pts = np.ascontiguousarray(points, dtype=np.float32).reshape(N, 3)
    freqs = (2.0 ** np.arange(NHARM)).astype(np.float32)
    fcol18 = (np.repeat(freqs[None, :], 3, axis=0).reshape(18, 1) / TWO_PI).astype(
        np.float32
    )

    bmat = np.zeros((128, 16), dtype=np.float32)
    for i in range(8):
        for m in range(2):
            bmat[:, i * 2 + m] = bs[i][m * 128:(m + 1) * 128]

    # packed layer-0 / layer-4-emb weights: M-half 0 at rows 0:39, M-half 1
    # at rows 64:103 (for row-tiled matmuls at array rows 0/64)
    w0p = np.zeros((128, 128), dtype=np.float16)
    w0p[0:E, :] = ws[0][:, 0:128].astype(np.float16)
    w0p[64:64 + E, :] = ws[0][:, 128:256].astype(np.float16)
    w4ep = np.zeros((128, 128), dtype=np.float16)
    w4ep[0:E, :] = ws[4][0:E, 0:128].astype(np.float16)
    w4ep[64:64 + E, :] = ws[4][0:E, 128:256].astype(np.float16)
    # consolidated fp16 weight wall (single DMA), layout must match _build:
    # w0p | w4ep | w{1,2,3,5,6,7}{a,b} | w4a | w4b | wsdf_a | wsdf_b
    parts = [w0p, w4ep]
    for i in (1, 2, 3, 5, 6, 7):
        parts.append(ws[i][0:128, :].astype(np.float16))
        parts.append(ws[i][128:256, :].astype(np.float16))
    parts.append(ws[4][E:E + 128, :].astype(np.float16))
    parts.append(ws[4][E + 128:E + 256, :].astype(np.float16))
    parts.append(wsdf[0:128, :].astype(np.float16))
    parts.append(wsdf[128:256, :].astype(np.float16))
    wallm = np.ascontiguousarray(np.concatenate(parts, axis=1))
    common = {
        "wallh": wallm,
        "bmat": bmat,
        "bsdf1": np.full((128, 1), float(np.ravel(bsdf)[0]), dtype=np.float32),
    }

    in_maps = []
    for c in range(N_CORES):
        sl = pts[c * NPC:(c + 1) * NPC]  # [NPC, 3]
        ptsT = np.ascontiguousarray(sl.T)  # [3, NPC]
        rep3 = np.repeat(ptsT, NHARM, axis=0)  # [18, NPC]
        t18 = rep3 * fcol18  # x * 2^j / (2pi), exact fp32 scaling
        # host-side range reduction to [-0.5, 0.5]: fp32 magic-number
        # round-to-nearest, same arithmetic the DVE trick would do
        mg = np.float32(MAGIC)
        ys_sin = t18 - ((t18 + mg) - mg)
        tc18 = t18 + np.float32(0.25)
        ys_cos = tc18 - ((tc18 + mg) - mg)
        rep6 = np.zeros((128, NPC), dtype=np.float32)
        rep6[0:18], rep6[18:36] = ys_sin, ys_cos
        rep6[64:82], rep6[82:100] = ys_sin, ys_cos
        m = dict(common)
        m["rep6"] = rep6
        m["ptsh"] = ptsT.astype(np.float16)
        in_maps.append(m)
    return in_maps


def kernel(
    points, w0, b0, w1, b1, w2, b2, w3, b3, w4, b4, w5, b5, w6, b6, w7, b7,
    wsdf, bsdf,
):
    ws = [np.asarray(w, dtype=np.float32) for w in (w0, w1, w2, w3, w4, w5, w6, w7)]
    bs = [np.asarray(b, dtype=np.float32) for b in (b0, b1, b2, b3, b4, b5, b6, b7)]
    in_maps = _prep_maps(
        np.asarray(points), ws, bs,
        np.asarray(wsdf, dtype=np.float32), np.asarray(bsdf, dtype=np.float32),
    )

    if "nc" not in _CACHED:
        _CACHED["nc"] = _build()
    nc = _CACHED["nc"]

    res = run_bass_kernel_spmd(nc, in_maps, core_ids=list(range(N_CORES)))
    out = np.concatenate(
        [res.results[c]["out_o"] for c in range(N_CORES)], axis=0
    ).reshape(N, 1).astype(np.float32)
    return out
